# revision 1
# baseline (speedup 1.0000x reference)
"""Trainium2 Bass kernel for nn_CustomBSplineLayer.

Math: out[b,o] = sum_{i,g} coeff[o,i,g] * w[o,i] * s_g(clip(x[b,i], -1, 1))
where s_g is a cubic B-spline basis on uniform knots (spacing h = 2/7,
centers linspace(-1, 15/7, 12), 8 basis functions, order 3; s_7 == 0 on the
clipped domain).

Uniform-knot truncated-power identity with t = (clip(x,-1,1)+1)/h in [0,7]:
    s_g = (1/6) * sum_{r=0..4} w5[r] * V_{g+r},   V_q = relu(t-q)^3,  q=0..6
so out = sum_{q,i} P_q[b,i] * H[(q,i), o] for ANY plane basis P that spans the
{V_q} (coefficients H solved exactly on host).  The PE runs float32r (full
rate; true fp32 is 4x slower) which rounds each product at ~2^-12 relative, so
per-plane error scales with |P_q|*|H_q|.  Raw V planes (|V|<=343) give ~1.2e-2
relative error; folding differences of neighbouring cubes bounds the planes:
    D1_q = V_q - V_{q+1} (<=127),   W2_q = D1_q - D1_{q+1} (<=36)
A mixed basis tuned per-plane to the measured error profile gives ~1.2e-3 at
only 8 extra subtracts per i-block:
    P = {W2_0, W2_1, W2_2, D1_3, D1_4, V_5, V_6}

Layout (data-parallel over batch, 8 cores x 1024 rows):
  - x pre-transposed on host: xt [512 i, 1024 b] per core, i on partitions.
  - planes per (i-block, q): [128, 1024] tiles; matmul lhsT slices are
    [128 K, 128 M=batch] column windows; rhs H tiles [128, 512 o] (f32r).
  - K order kt = ib*7 + q matches plane production order.
  - PSUM [128 b, 512 o] x 8 banks accumulate all 28 k-tiles.
  - Engine split: ScalarE does relu+square (+psum drains), VectorE does cubes
    and f32r-facing folds/casts, GpSimd does interior fp32 D1 folds.
  - DVE ops READING float32r tiles round to ~12 bits (measured), so every
    fold input stays fp32; f32r appears only on op OUTPUTS (bit-benign) or
    via cheap copy-casts.
"""

import numpy as np

import concourse.mybir as mybir
from concourse import bacc
import concourse.tile as tile
from concourse.bass_utils import run_bass_kernel_spmd

F32 = mybir.dt.float32
F32R = mybir.dt.float32r
AOT = mybir.AluOpType
ACTF = mybir.ActivationFunctionType

N_CORES = 8
BATCH, I, O, G = 8192, 512, 512, 8
BC = BATCH // N_CORES          # 1024 batch rows per core
Q = 7                          # planes q = 0..6
IB = I // 128                  # 4 i-blocks
KT = Q * IB                    # 28 k-tiles of 128
NBB = BC // 128                # 8 batch blocks of 128

# plane level per q: 0 = raw V, 1 = D1, 2 = W2
LEVELS = (2, 2, 2, 1, 1, 0, 0)
SQ_DVE = 6         # how many of the 28 squares run on DVE (rest ScalarE)

_programs = {}


def _build_program(knobs=(LEVELS, SQ_DVE)):
    levels, sq_dve = knobs
    nc = bacc.Bacc("TRN2", target_bir_lowering=False, debug=False,
                   num_devices=N_CORES)
    xt_d = nc.dram_tensor("xt", [I, BC], F32, kind="ExternalInput").ap()
    h2_d = nc.dram_tensor("h2", [KT * 128, O], F32R, kind="ExternalInput").ap()
    qb_d = nc.dram_tensor("qb", [128, 8], F32, kind="ExternalInput").ap()
    out_d = nc.dram_tensor("out", [BC, O], F32, kind="ExternalOutput").ap()

    assert levels == (2, 2, 2, 1, 1, 0, 0), "kernel body is specialized"

    with tile.TileContext(nc) as tc:
        with tc.tile_pool(name="g", bufs=1) as gpool, \
             tc.tile_pool(name="x", bufs=2) as xpool, \
             tc.tile_pool(name="v", bufs=1) as vpool, \
             tc.tile_pool(name="tmp", bufs=3) as tpool, \
             tc.tile_pool(name="o", bufs=4) as opool, \
             tc.tile_pool(name="ps", bufs=1, space="PSUM") as pspool:

            qb_s = gpool.tile([128, 8], F32)
            nc.sync.dma_start(out=qb_s[:], in_=qb_d[:])

            h2_s = gpool.tile([128, KT, O], F32R)
            for ib in range(IB):
                nc.sync.dma_start(
                    out=h2_s[:, ib * Q:(ib + 1) * Q, :],
                    in_=h2_d[ib * Q * 128:(ib + 1) * Q * 128, :].rearrange(
                        "(kt p) o -> p kt o", p=128))

            psums = [pspool.tile([128, O], F32, name=f"ps{bb}", tag=f"ps{bb}")
                     for bb in range(NBB)]

            state = {"sq_dve": sq_dve}

            def mk_plane(ib, q, lhs):
                kt = ib * Q + q
                rhs = h2_s[:, kt, :]
                for bb in range(NBB):
                    nc.tensor.matmul(psums[bb][:],
                                     lhs[:, bb * 128:(bb + 1) * 128],
                                     rhs,
                                     start=(kt == 0), stop=(kt == KT - 1))

            for ib in range(IB):
                xs = xpool.tile([128, BC], F32, tag="x")
                nc.sync.dma_start(out=xs[:], in_=xt_d[ib * 128:(ib + 1) * 128, :])
                tp = xpool.tile([128, BC], F32, tag="tp")
                nc.vector.tensor_scalar(out=tp[:], in0=xs[:], scalar1=3.5,
                                        scalar2=3.5, op0=AOT.mult, op1=AOT.min)

                v = {}
                d1 = {}

                def cube(q, dtype=F32, tag="v", bufs=4):
                    qq = float(q) - 3.5
                    a = tpool.tile([128, BC], F32, tag="a")
                    nc.scalar.activation(a[:], tp[:], ACTF.Relu,
                                         bias=qb_s[:, q:q + 1], scale=1.0)
                    sq = tpool.tile([128, BC], F32, tag="sq")
                    if state["sq_dve"] > 0 and q == 3:
                        state["sq_dve"] -= 1
                        nc.vector.scalar_tensor_tensor(
                            out=sq[:], in0=tp[:], scalar=qq, in1=a[:],
                            op0=AOT.subtract, op1=AOT.mult)
                    else:
                        nc.scalar.activation(sq[:], a[:], ACTF.Square)
                    vq = vpool.tile([128, BC], dtype, tag=tag, bufs=bufs,
                                    name=f"{tag}_{ib}_{q}")
                    nc.vector.scalar_tensor_tensor(
                        out=vq[:], in0=tp[:], scalar=qq, in1=sq[:],
                        op0=AOT.subtract, op1=AOT.mult)
                    return vq

                def gp_sub(name, x0, x1):
                    dq = vpool.tile([128, BC], F32, tag="d1", bufs=3,
                                    name=f"{name}_{ib}")
                    nc.gpsimd.tensor_tensor(out=dq[:], in0=x0[:], in1=x1[:],
                                            op=AOT.subtract)
                    return dq

                def dve_sub_r(name, x0, x1):
                    wq = vpool.tile([128, BC], F32R, tag="w2", bufs=4,
                                    name=f"{name}_{ib}")
                    nc.vector.tensor_tensor(out=wq[:], in0=x0[:], in1=x1[:],
                                            op=AOT.subtract)
                    return wq

                for q in range(2):
                    v[q] = cube(q)
                d1[0] = gp_sub("d1_0", v[0], v[1])
                v[2] = cube(2)
                d1[1] = gp_sub("d1_1", v[1], v[2])
                mk_plane(ib, 0, dve_sub_r("w2_0", d1[0], d1[1]))
                v[3] = cube(3)
                d1[2] = gp_sub("d1_2", v[2], v[3])
                mk_plane(ib, 1, dve_sub_r("w2_1", d1[1], d1[2]))
                v[4] = cube(4)
                d1[3] = gp_sub("d1_3", v[3], v[4])
                mk_plane(ib, 2, dve_sub_r("w2_2", d1[2], d1[3]))
                # plane 3 = D1_3 (f32r copy-cast of the fp32 fold output)
                p3 = vpool.tile([128, BC], F32R, tag="w2", bufs=4,
                                name=f"p3_{ib}")
                nc.vector.tensor_copy(out=p3[:], in_=d1[3][:])
                mk_plane(ib, 3, p3)
                v[5] = cube(5)
                # plane 4 = D1_4 = v4 - v5 (f32r out directly; plane-only)
                mk_plane(ib, 4, dve_sub_r("d1_4", v[4], v[5]))
                # plane 5 = V_5 (f32r copy-cast)
                p5 = vpool.tile([128, BC], F32R, tag="w2", bufs=4,
                                name=f"p5_{ib}")
                nc.vector.tensor_copy(out=p5[:], in_=v[5][:])
                mk_plane(ib, 5, p5)
                # plane 6 = V_6, cube written straight to f32r
                mk_plane(ib, 6, cube(6, dtype=F32R, tag="w2", bufs=4))

            for bb in range(NBB):
                o = opool.tile([128, O], F32, tag="o")
                nc.scalar.copy(o[:], psums[bb][:])
                nc.sync.dma_start(out=out_d[bb * 128:(bb + 1) * 128, :], in_=o[:])

    nc.compile()
    return nc


def _get_program(knobs=(LEVELS, SQ_DVE)):
    if knobs not in _programs:
        _programs[knobs] = _build_program(knobs)
    return _programs[knobs]


_STENS = {0: (1.0,), 1: (1.0, -1.0), 2: (1.0, -2.0, 1.0)}


def _host_prep(x, weights, coefficients, levels=LEVELS):
    x = np.ascontiguousarray(np.asarray(x, dtype=np.float32))
    weights = np.asarray(weights, dtype=np.float32)
    coefficients = np.asarray(coefficients, dtype=np.float32)

    # raw truncated-power coefficients G_q = sum_g w5[q-g]/6 * C2_g  (g<=6)
    c2 = coefficients.astype(np.float64) * weights.astype(np.float64)[:, :, None]
    c2 = c2.transpose(2, 1, 0)[:Q]                 # [7, I, O]
    w5 = np.array([1.0, -4.0, 6.0, -4.0, 1.0])
    graw = np.zeros((Q, I, O), dtype=np.float64)
    for q in range(Q):
        for g in range(Q):
            r = q - g
            if 0 <= r <= 4:
                graw[q] += (w5[r] / 6.0) * c2[g]
    # planes P = A V  =>  coefficients H = A^{-T} G (exact basis change)
    A = np.zeros((Q, Q))
    for q in range(Q):
        for u, s in enumerate(_STENS[levels[q]]):
            if q + u < Q:
                A[q, q + u] = s
    h = np.einsum('pq,qio->pio', np.linalg.inv(A).T, graw)
    # device row order kt = ib*7 + q
    h2k = np.empty((KT, 128, O), dtype=np.float32)
    for ib in range(IB):
        for q in range(Q):
            h2k[ib * Q + q] = h[q, ib * 128:(ib + 1) * 128, :]
    h2k = np.ascontiguousarray(h2k.reshape(KT * 128, O))

    xt = np.ascontiguousarray(x.T)                 # [I, B]
    qb = np.tile((3.5 - np.arange(8, dtype=np.float32))[None, :], (128, 1))

    in_maps = []
    for c in range(N_CORES):
        in_maps.append({
            "xt": np.ascontiguousarray(xt[:, c * BC:(c + 1) * BC]),
            "h2": h2k,
            "qb": qb,
        })
    return in_maps


def _run(x, weights, coefficients, knobs=(LEVELS, SQ_DVE), **spmd_kwargs):
    nc = _get_program(knobs)
    in_maps = _host_prep(x, weights, coefficients, knobs[0])
    res = run_bass_kernel_spmd(nc, in_maps, list(range(N_CORES)), **spmd_kwargs)
    out = np.concatenate([res.results[c]["out"] for c in range(N_CORES)], axis=0)
    return out.astype(np.float32), res


def kernel(x, weights, coefficients):
    out, _ = _run(x, weights, coefficients)
    return out



# revision 3
# speedup vs baseline: 1.5020x; 1.5020x over previous
"""Trainium2 Bass kernel for nn_CustomBSplineLayer.

Math: out[b,o] = sum_{i,g} coeff[o,i,g] * w[o,i] * s_g(clip(x[b,i], -1, 1))
where s_g is a cubic B-spline basis (integer knots in t = 3.5*(x+1) space).

Truncated-power identity: V_q = relu(t-q)^3 (q=0..6) spans all s_g on [0,7],
so out = sum_{q,i} P_q[b,i] * H[(q,i), o] for any plane basis P spanning
{V_q} (H solved exactly on host).  Plane basis (levels 2,2,1,0,0,0,0):
    P = {W2_0, W2_1, D1_2, V_3, V_4, V_5, V_6}     (max magnitudes
    36, 30, 61, 64, 27, 8, 1)
with D1_q = V_q - V_{q+1}, W2_q = D1_q - D1_{q+1}.

Precision design (gate is 2e-2; this lands ~4.7e-3 in exact simulation):
  - planes and H are fp16: fp16 x fp16 matmul products are EXACT in f32
    PSUM (PE full rate, same as f32r, minus f32r's 2^-12 product rounding).
  - the fold chain q<=3 (big cube values, catastrophic cancellation if
    quantized early) stays fp32 end-to-end; only the final small-magnitude
    plane values round to fp16.
  - the q>=4 cubes run all-fp16 (V = a*a*a with a = relu(t-q) small), which
    makes those DVE ops 2-4x faster (dve 2x/4x perf modes need 2-byte
    dtypes).
  - tpc = clip(3.5 x, -3.5, 3.5) fp16 on host; its quantization enters all
    planes through the same delta-t, and every plane has small d/dt.

Engine split (per i-block): ACT does 7 relus + 2 squares + 1 cast copy,
DVE does 2 f32 squares + 4 f32 cubes + 2 f32->f16 fold subs + 1 cast +
6 fp16 square/cube ops, GpSimd does the 3 f32 D1 subs.  PSUM drains
alternate ACT/DVE.  Production order q = 5,6,4,3,2,1,0 puts cheap fp16
planes first so the PE starts ~2.5us in and then never starves — the PE
only reaches its full 2.4 GHz pstate after ~3us of CONTINUOUS execution,
so any gap halves matmul throughput.

Layout (data-parallel over batch, 8 cores x 1024 rows):
  - xt [512 i, 1024 b] fp16 per core (i on partitions), relu bias 3.5-q.
  - planes per (i-block, q): [128, 1024] fp16; matmul lhsT slices are
    [128 K, 128 M=batch]; rhs H tiles [128, 512 o] fp16.
  - PSUM [128 b, 512 o] f32 x 8 banks accumulate all 28 k-tiles; each bank
    drains right after its final matmul.
"""

import numpy as np

import concourse.mybir as mybir
from concourse import bacc
import concourse.tile as tile
from concourse.bass_utils import run_bass_kernel_spmd

F32 = mybir.dt.float32
F16 = mybir.dt.float16
AOT = mybir.AluOpType
ACTF = mybir.ActivationFunctionType

N_CORES = 8
BATCH, I, O, G = 8192, 512, 512, 8
BC = BATCH // N_CORES          # 1024 batch rows per core
Q = 7                          # planes q = 0..6
IB = I // 128                  # 4 i-blocks
KT = Q * IB                    # 28 k-tiles of 128
NBB = BC // 128                # 8 batch blocks of 128

LEVELS = (2, 2, 1, 0, 0, 0, 0)

_programs = {}


def _build_program(knobs=0):
    nc = bacc.Bacc("TRN2", target_bir_lowering=False, debug=False,
                   num_devices=N_CORES)
    xt_d = nc.dram_tensor("xt", [I, BC], F16, kind="ExternalInput").ap()
    h2_d = nc.dram_tensor("h2", [KT * 128, O], F16, kind="ExternalInput").ap()
    qb_d = nc.dram_tensor("qb", [128, 8], F32, kind="ExternalInput").ap()
    out_d = nc.dram_tensor("out", [BC, O], F32, kind="ExternalOutput").ap()

    with tile.TileContext(nc) as tc:
        with tc.tile_pool(name="g", bufs=1) as gpool, \
             tc.tile_pool(name="x", bufs=2) as xpool, \
             tc.tile_pool(name="af", bufs=5) as afpool, \
             tc.tile_pool(name="ah", bufs=3) as ahpool, \
             tc.tile_pool(name="sf", bufs=5) as sfpool, \
             tc.tile_pool(name="sh", bufs=3) as shpool, \
             tc.tile_pool(name="vf", bufs=6) as vfpool, \
             tc.tile_pool(name="df", bufs=4) as dfpool, \
             tc.tile_pool(name="p", bufs=12) as ppool, \
             tc.tile_pool(name="o", bufs=4) as opool, \
             tc.tile_pool(name="ps", bufs=1, space="PSUM") as pspool:

            qb_s = gpool.tile([128, 8], F32)
            nc.sync.dma_start(out=qb_s[:], in_=qb_d[:])

            h2_s = gpool.tile([128, KT, O], F16)
            for ib in range(IB):
                nc.sync.dma_start(
                    out=h2_s[:, ib * Q:(ib + 1) * Q, :],
                    in_=h2_d[ib * Q * 128:(ib + 1) * Q * 128, :].rearrange(
                        "(kt p) o -> p kt o", p=128))

            psums = [pspool.tile([128, O], F32, name=f"ps{bb}", tag=f"ps{bb}")
                     for bb in range(NBB)]

            issue = {"n": 0}

            def mk_plane(ib, q, lhs):
                kt = ib * Q + q
                rhs = h2_s[:, kt, :]
                first = issue["n"] == 0
                last = issue["n"] == KT - 1
                issue["n"] += 1
                for bb in range(NBB):
                    nc.tensor.matmul(psums[bb][:],
                                     lhs[:, bb * 128:(bb + 1) * 128],
                                     rhs,
                                     start=first, stop=last)
                    if last:
                        o = opool.tile([128, O], F32, tag="o")
                        if bb % 2 == 0:
                            nc.scalar.copy(o[:], psums[bb][:])
                        else:
                            nc.vector.tensor_copy(out=o[:], in_=psums[bb][:])
                        nc.sync.dma_start(
                            out=out_d[bb * 128:(bb + 1) * 128, :], in_=o[:])

            for ib in range(IB):
                xs = xpool.tile([128, BC], F16, tag="x")
                nc.sync.dma_start(out=xs[:], in_=xt_d[ib * 128:(ib + 1) * 128, :])

                def relu(q, dtype):
                    a = (afpool if dtype == F32 else ahpool).tile(
                        [128, BC], dtype, tag="a" + ("f" if dtype == F32 else "h"))
                    nc.scalar.activation(a[:], xs[:], ACTF.Relu,
                                         bias=qb_s[:, q:q + 1], scale=1.0)
                    return a

                def cube16(q):
                    """fp16 V_q = a*a*a (values small for q>=4)."""
                    a = relu(q, F16)
                    sq = shpool.tile([128, BC], F16, tag="sh")
                    nc.vector.tensor_tensor(out=sq[:], in0=a[:], in1=a[:],
                                            op=AOT.mult)
                    vq = ppool.tile([128, BC], F16, tag="pl", bufs=12,
                                    name=f"v16_{ib}_{q}")
                    nc.vector.tensor_tensor(out=vq[:], in0=sq[:], in1=a[:],
                                            op=AOT.mult)
                    return vq

                def cube32(q, sq_on_act):
                    """fp32 V_q for the fold chain (exact differences)."""
                    a = relu(q, F32)
                    sq = sfpool.tile([128, BC], F32, tag="sf")
                    if sq_on_act:
                        nc.scalar.activation(sq[:], a[:], ACTF.Square)
                    else:
                        nc.vector.tensor_tensor(out=sq[:], in0=a[:], in1=a[:],
                                                op=AOT.mult)
                    vq = vfpool.tile([128, BC], F32, tag="vf", bufs=6,
                                     name=f"v32_{ib}_{q}")
                    nc.vector.tensor_tensor(out=vq[:], in0=sq[:], in1=a[:],
                                            op=AOT.mult)
                    return vq

                # cheap fp16 planes first so the PE starts early
                mk_plane(ib, 5, cube16(5))
                mk_plane(ib, 6, cube16(6))
                mk_plane(ib, 4, cube16(4))
                # fp32 fold chain: v3, v2 first (d1_2 gates the GP chain)
                v3 = cube32(3, sq_on_act=False)
                p3 = ppool.tile([128, BC], F16, tag="pl", bufs=12,
                                name=f"p3_{ib}")
                nc.scalar.copy(p3[:], v3[:])
                mk_plane(ib, 3, p3)
                v2 = cube32(2, sq_on_act=False)
                d1_2 = dfpool.tile([128, BC], F32, tag="df", name=f"d1_2_{ib}")
                nc.gpsimd.tensor_tensor(out=d1_2[:], in0=v2[:], in1=v3[:],
                                        op=AOT.subtract)
                p2 = ppool.tile([128, BC], F16, tag="pl", bufs=12,
                                name=f"p2_{ib}")
                nc.vector.tensor_copy(out=p2[:], in_=d1_2[:])
                mk_plane(ib, 2, p2)
                v1 = cube32(1, sq_on_act=True)
                d1_1 = dfpool.tile([128, BC], F32, tag="df", name=f"d1_1_{ib}")
                nc.gpsimd.tensor_tensor(out=d1_1[:], in0=v1[:], in1=v2[:],
                                        op=AOT.subtract)
                p1 = ppool.tile([128, BC], F16, tag="pl", bufs=12,
                                name=f"p1_{ib}")
                nc.vector.tensor_tensor(out=p1[:], in0=d1_1[:], in1=d1_2[:],
                                        op=AOT.subtract)
                mk_plane(ib, 1, p1)
                v0 = cube32(0, sq_on_act=True)
                d1_0 = dfpool.tile([128, BC], F32, tag="df", name=f"d1_0_{ib}")
                nc.gpsimd.tensor_tensor(out=d1_0[:], in0=v0[:], in1=v1[:],
                                        op=AOT.subtract)
                p0 = ppool.tile([128, BC], F16, tag="pl", bufs=12,
                                name=f"p0_{ib}")
                nc.vector.tensor_tensor(out=p0[:], in0=d1_0[:], in1=d1_1[:],
                                        op=AOT.subtract)
                mk_plane(ib, 0, p0)

    nc.compile()
    return nc


def _get_program(knobs=0):
    if knobs not in _programs:
        _programs[knobs] = _build_program(knobs)
    return _programs[knobs]


_STENS = {0: (1.0,), 1: (1.0, -1.0), 2: (1.0, -2.0, 1.0)}


def _host_prep(x, weights, coefficients, levels=LEVELS):
    x = np.ascontiguousarray(np.asarray(x, dtype=np.float32))
    weights = np.asarray(weights, dtype=np.float32)
    coefficients = np.asarray(coefficients, dtype=np.float32)

    # raw truncated-power coefficients G_q = sum_g w5[q-g]/6 * C2_g  (g<=6)
    c2 = coefficients.astype(np.float64) * weights.astype(np.float64)[:, :, None]
    c2 = c2.transpose(2, 1, 0)[:Q]                 # [7, I, O]
    w5 = np.array([1.0, -4.0, 6.0, -4.0, 1.0])
    graw = np.zeros((Q, I, O), dtype=np.float64)
    for q in range(Q):
        for g in range(Q):
            r = q - g
            if 0 <= r <= 4:
                graw[q] += (w5[r] / 6.0) * c2[g]
    # planes P = A V  =>  coefficients H = A^{-T} G (exact basis change)
    A = np.zeros((Q, Q))
    for q in range(Q):
        for u, s in enumerate(_STENS[levels[q]]):
            if q + u < Q:
                A[q, q + u] = s
    h = np.einsum('pq,qio->pio', np.linalg.inv(A).T, graw)
    # device row order kt = ib*7 + q
    h2k = np.empty((KT, 128, O), dtype=np.float16)
    for ib in range(IB):
        for q in range(Q):
            h2k[ib * Q + q] = h[q, ib * 128:(ib + 1) * 128, :]
    h2k = np.ascontiguousarray(h2k.reshape(KT * 128, O))

    # tpc = clip(3.5*x, -3.5, 3.5) in t-minus-3.5 coords; relu bias is 3.5-q
    tpc = np.clip(3.5 * x, -3.5, 3.5).astype(np.float16)
    xt = np.ascontiguousarray(tpc.T)               # [I, B] fp16
    qb = np.tile((3.5 - np.arange(8, dtype=np.float32))[None, :], (128, 1))

    in_maps = []
    for c in range(N_CORES):
        in_maps.append({
            "xt": np.ascontiguousarray(xt[:, c * BC:(c + 1) * BC]),
            "h2": h2k,
            "qb": qb,
        })
    return in_maps


def _run(x, weights, coefficients, knobs=0, **spmd_kwargs):
    nc = _get_program(knobs)
    in_maps = _host_prep(x, weights, coefficients)
    res = run_bass_kernel_spmd(nc, in_maps, list(range(N_CORES)), **spmd_kwargs)
    out = np.concatenate([res.results[c]["out"] for c in range(N_CORES)], axis=0)
    return out.astype(np.float32), res


def kernel(x, weights, coefficients):
    out, _ = _run(x, weights, coefficients)
    return out
